# revision 31
# baseline (speedup 1.0000x reference)
"""GCN (3-layer GCNConv + mean-pool + MLP head) Trainium2 Bass kernel, 8 NeuronCores.

Strategy (graph/data parallel, per sharding hint):
  - Destination nodes are partitioned into 8 contiguous blocks (one per core).
  - Host partitions the edge list (self-loops included as ordinary edges with
    norm 1/deg) by destination block, then by destination window (128 dst
    nodes per window) and by source-range group (4 groups of 25088 table rows,
    so gather indices fit int16), laying edges out on a [128, COLS] grid.
  - Per layer, each core bulk-gathers source rows with InstDMAGatherAnt
    (dma_gather): one call per (chunk of 7 windows x source group), thousands
    of rows per call, from a 256B-row padded node table in DRAM. This is
    ~2x cheaper per row in GPSIMD descriptor-generation time than per-window
    indirect DMAs and amortizes instruction overhead ~40x.
  - Messages are scaled by the GCN edge norm (one vector op per call) and
    segment-summed into destination windows with one-hot x message matmuls
    accumulated in PSUM (aggregate-then-transform).
  - Per-shard layer output (relu(agg @ W + b)) is written into a padded
    [SHARD, 128] bounce buffer and AllGathered into the next layer's table.
  - Layer 3 output is mean-pooled per graph locally (one-hot matmul into a
    PSUM accumulator), AllReduced across cores, and the tiny FC head runs
    replicated on every core.
"""

import os
import sys
from dataclasses import dataclass

import numpy as np
import ml_dtypes

for _p in ("/opt/trn_rl_repo", "/root/.axon_site/_ro/trn_rl_repo"):
    if os.path.isdir(_p) and _p not in sys.path:
        sys.path.insert(0, _p)

bf16 = ml_dtypes.bfloat16
P = 128


@dataclass
class GCNConfig:
    N: int = 100000          # real nodes
    G: int = 128             # graphs (output width; PSUM col budget)
    SHARD: int = 12544       # padded nodes per core (NW * 128)
    NW: int = 98             # dst windows per core
    NG: int = 4              # source-range groups (int16 gather index limit)
    TG: int = 5              # edge columns per (window, group); auto-derived
    CW: int = 7              # windows per gather chunk
    F: tuple = (40, 40, 80, 160)   # feature dims x -> h1 -> h2 -> h3
    HID: int = 128           # fc hidden
    n_cores: int = 8

    @property
    def NPAD(self):
        return self.n_cores * self.SHARD

    @property
    def GRP(self):
        return self.NPAD // self.NG

    @property
    def COLS(self):
        # column c of the grid: call (ch, g) covers CW*TG contiguous columns;
        # global col = ((ch*NG + g)*CW + wloc)*TG + t
        return self.NW * self.NG * self.TG

    @property
    def NCH(self):
        return self.NW // self.CW


CFG = GCNConfig()


# ---------------------------------------------------------------- host prep

def build_host_data(cfg, inp):
    """Partition/sort edges (incl self-loops) by (dst window, src group),
    compute GCN edge norms, build per-core gather-index + norm/slot grids."""
    N, SHARD, NW, NG, TG, CW = cfg.N, cfg.SHARD, cfg.NW, cfg.NG, cfg.TG, cfg.CW
    GRP = cfg.GRP
    src = np.asarray(inp["edge_index"][0]).astype(np.int64).ravel()
    dst = np.asarray(inp["edge_index"][1]).astype(np.int64).ravel()
    batch = np.asarray(inp["batch"]).astype(np.int64).ravel()
    deg = (np.bincount(dst, minlength=N) + 1).astype(np.float32)
    dis = 1.0 / np.sqrt(deg)
    # self-loops (weight 1/deg) are applied via a dense per-window diagonal
    # term on the local shard, not via the gather grid
    srcA, dstA = src, dst
    norm = (dis[src] * dis[dst]).astype(np.float32)

    # segment-major table layout: two AllGather segments per layer boundary.
    SEG = SHARD // 2
    vv = np.arange(cfg.NPAD, dtype=np.int64)
    vc, vr = vv // SHARD, vv % SHARD
    vs = vr // SEG
    remap = vs * (cfg.n_cores * SEG) + vc * SEG + (vr - vs * SEG)

    srcR = remap[srcA]               # remapped source row in the padded table
    grp = srcR // GRP                # source group (gather call bucket)
    core = dstA // SHARD
    win = (dstA % SHARD) // P
    dloc = (dstA % SHARD) % P
    order = np.lexsort((grp, win, core))
    srcR, grp, core, win, dloc, norm = (
        a[order] for a in (srcR, grp, core, win, dloc, norm))

    cores = []
    for c in range(cfg.n_cores):
        m = core == c
        sr, gr, wn, dl, nm = srcR[m], grp[m], win[m], dloc[m], norm[m]
        esrc16 = np.zeros((16, cfg.COLS * P // 16), np.int16)
        edl = np.full((P, cfg.COLS), -1.0, bf16)
        enrm = np.zeros((P, cfg.COLS), bf16)
        # edges sorted by (win, grp); per (win, grp) cell: slot j ->
        # global col = ((ch*NG + g)*CW + wloc)*TG + j//128, p = j%128
        cell = wn * NG + gr                      # 0 .. NW*NG-1
        ccnt = np.bincount(cell, minlength=NW * NG)
        assert ccnt.max() <= TG * P, f"cell overflow: {ccnt.max()} > {TG * P}"
        jin = (np.concatenate([np.arange(n) for n in ccnt])
               if len(cell) else np.zeros(0, np.int64))
        ch = wn // CW
        wloc = wn % CW
        col = ((ch * NG + gr) * CW + wloc) * TG + jin // P
        pp = jin % P
        ipos = col * P + pp                      # dma_gather index position
        esrc16[ipos % 16, ipos // 16] = (sr - gr * GRP).astype(np.int16)
        edl[pp, col] = dl.astype(bf16)
        enrm[pp, col] = nm.astype(bf16)
        nid = np.arange(SHARD) + c * SHARD
        gl = np.where(nid < N, batch[np.minimum(nid, N - 1)], -1).astype(np.float32)
        gloc = np.ascontiguousarray(gl.reshape(NW, P).T).astype(bf16)
        d2 = np.where(nid < N, 1.0 / deg[np.minimum(nid, N - 1)], 0.0).astype(np.float32)
        dis2 = np.ascontiguousarray(d2.reshape(NW, P).T).astype(bf16)
        xs = np.zeros((SHARD, cfg.F[0]), bf16)
        nreal = max(0, min(SHARD, N - c * SHARD))
        xs[:nreal] = np.asarray(inp["x"])[c * SHARD:c * SHARD + nreal].astype(bf16)
        cores.append(dict(esrc16=np.tile(esrc16, (8, 1)), edl=edl, enrm=enrm,
                          gloc=gloc, dis2=dis2, xs=xs))

    xt = np.zeros((cfg.NPAD, P), bf16)
    xt[remap[:N], :cfg.F[0]] = np.asarray(inp["x"]).astype(bf16)

    cnt = np.bincount(batch, minlength=cfg.G).astype(np.float32)
    invc = np.zeros((P, 1), np.float32)
    invc[:cfg.G, 0] = 1.0 / np.maximum(cnt, 1.0)

    def a2(x, dt):
        return np.ascontiguousarray(np.asarray(x), dtype=dt)

    wts = dict(
        w1a=np.concatenate([a2(inp["W1"], bf16), a2(inp["b1"], bf16)[None]], 0),
        w2a=np.concatenate([a2(inp["W2"], bf16), a2(inp["b2"], bf16)[None]], 0),
        w3a=np.concatenate([a2(inp["W3"], bf16), a2(inp["b3"], bf16)[None]], 0),
        fw1=a2(inp["fW1"], bf16),
        fb1c=a2(inp["fb1"], np.float32).reshape(-1, 1),
        fw2=a2(inp["fW2"], bf16),
        invc=invc,
    )
    fb2 = float(np.asarray(inp["fb2"]).ravel()[0])
    return cores, xt, wts, fb2


# ---------------------------------------------------------------- bass build

def build_bass(cfg, fb2):
    import concourse.bacc as bacc
    import concourse.bass as bass
    import concourse.mybir as mybir
    import concourse.tile as tile
    from concourse.masks import make_identity

    dt = mybir.dt
    AF = mybir.ActivationFunctionType
    OP = mybir.AluOpType
    F0, F1, F2, F3 = cfg.F
    NW, NG, TG, CW, NCH, G = cfg.NW, cfg.NG, cfg.TG, cfg.CW, cfg.NCH, cfg.G
    GRP = cfg.GRP
    FMAX = max(F0, F1, F2)
    CALL_COLS = CW * TG
    CALL_IDX = CALL_COLS * P
    NQ = 4

    nc = bacc.Bacc("TRN2", target_bir_lowering=False, debug=False,
                   enable_asserts=False, num_devices=cfg.n_cores,
                   num_swdge_queues=NQ)

    # ---- I/O
    xt_d = nc.dram_tensor("xt", [cfg.NPAD, P], dt.bfloat16, kind="ExternalInput")
    esrc_d = nc.dram_tensor("esrc16", [P, cfg.COLS * P // 16], dt.int16,
                            kind="ExternalInput")
    edl_d = nc.dram_tensor("edl", [P, cfg.COLS], dt.bfloat16, kind="ExternalInput")
    enrm_d = nc.dram_tensor("enrm", [P, cfg.COLS], dt.bfloat16, kind="ExternalInput")
    gloc_d = nc.dram_tensor("gloc", [P, NW], dt.bfloat16, kind="ExternalInput")
    dis2_d = nc.dram_tensor("dis2", [P, NW], dt.bfloat16, kind="ExternalInput")
    xs_d = nc.dram_tensor("xs", [cfg.SHARD, F0], dt.bfloat16, kind="ExternalInput")
    w1a_d = nc.dram_tensor("w1a", [F0 + 1, F1], dt.bfloat16, kind="ExternalInput")
    w2a_d = nc.dram_tensor("w2a", [F1 + 1, F2], dt.bfloat16, kind="ExternalInput")
    w3a_d = nc.dram_tensor("w3a", [F2 + 1, F3], dt.bfloat16, kind="ExternalInput")
    fw1_d = nc.dram_tensor("fw1", [F3, cfg.HID], dt.bfloat16, kind="ExternalInput")
    fb1_d = nc.dram_tensor("fb1c", [cfg.HID, 1], dt.float32, kind="ExternalInput")
    fw2_d = nc.dram_tensor("fw2", [cfg.HID, 1], dt.bfloat16, kind="ExternalInput")
    invc_d = nc.dram_tensor("invc", [P, 1], dt.float32, kind="ExternalInput")
    out_d = nc.dram_tensor("out", [1, P], dt.float32, kind="ExternalOutput")

    rg = [list(range(cfg.n_cores))]

    with tile.TileContext(nc) as tc:
        with (
            tc.tile_pool(name="res", bufs=1) as res,                  # persistent SBUF
            tc.tile_pool(name="fat", bufs=8) as fatp,
            tc.tile_pool(name="mw", bufs=5) as mwp,
            tc.tile_pool(name="sp", bufs=6) as sp,
            tc.tile_pool(name="work", bufs=2) as work,
            tc.tile_pool(name="pa_ps", bufs=1, space="PSUM") as pa_ps,
            tc.tile_pool(name="p2_ps", bufs=1, space="PSUM") as p2_ps,
            tc.tile_pool(name="pool_ps", bufs=1, space="PSUM") as pool_ps,
            tc.tile_pool(name="head_ps", bufs=1, space="PSUM") as head_ps,
            tc.tile_pool(name="dram", bufs=1, space="DRAM") as dram,
        ):
            # ---- load persistent SBUF state
            esrc16 = res.tile([P, cfg.COLS * P // 16], dt.int16)
            edl = res.tile([P, cfg.COLS], dt.bfloat16)
            enrm = res.tile([P, cfg.COLS], dt.bfloat16)
            gloc = res.tile([P, NW], dt.bfloat16)
            dis2 = res.tile([P, NW], dt.bfloat16)
            w1a = res.tile([F0 + 1, F1], dt.bfloat16)
            w2a = res.tile([F1 + 1, F2], dt.bfloat16)
            w3a = res.tile([F2 + 1, F3], dt.bfloat16)
            fw1a = res.tile([F3 // 2, cfg.HID], dt.bfloat16)
            fw1b = res.tile([F3 // 2, cfg.HID], dt.bfloat16)
            fb1c = res.tile([cfg.HID, 1], dt.float32)
            fw2 = res.tile([cfg.HID, 1], dt.bfloat16)
            invc = res.tile([P, 1], dt.float32)
            b1r = res.tile([1, F1], dt.bfloat16)
            b2r = res.tile([1, F2], dt.bfloat16)
            b3r = res.tile([1, F3], dt.bfloat16)
            for sb, dr in ((esrc16, esrc_d), (edl, edl_d), (enrm, enrm_d),
                           (gloc, gloc_d), (dis2, dis2_d), (w1a, w1a_d), (w2a, w2a_d),
                           (w3a, w3a_d), (fb1c, fb1_d), (fw2, fw2_d),
                           (invc, invc_d)):
                nc.sync.dma_start(out=sb[:], in_=dr[:])
            nc.sync.dma_start(out=b1r[:], in_=w1a_d[F0:F0 + 1, :])
            nc.sync.dma_start(out=b2r[:], in_=w2a_d[F1:F1 + 1, :])
            nc.sync.dma_start(out=b3r[:], in_=w3a_d[F2:F2 + 1, :])
            nc.sync.dma_start(out=fw1a[:], in_=fw1_d[0:F3 // 2, :])
            nc.sync.dma_start(out=fw1b[:], in_=fw1_d[F3 // 2:, :])

            iota_i = res.tile([P, P], dt.int32)
            nc.gpsimd.iota(iota_i[:], pattern=[[1, P]], base=0, channel_multiplier=0)
            iota_b = res.tile([P, P], dt.bfloat16)
            nc.vector.tensor_copy(out=iota_b[:], in_=iota_i[:])
            ident = res.tile([P, P], dt.bfloat16)
            make_identity(nc, ident[:])
            ones1 = res.tile([1, P], dt.bfloat16)
            nc.vector.memset(ones1[:], 1.0)

            # ---- DRAM tables / bounce buffers (256B-row padded)
            h1s = dram.tile([cfg.SHARD, P], dt.bfloat16)
            h2s = dram.tile([cfg.SHARD, P], dt.bfloat16)
            h1t = dram.tile([cfg.NPAD, P], dt.bfloat16)
            h2t = dram.tile([cfg.NPAD, P], dt.bfloat16)
            pool_pt = dram.tile([P, F3], dt.float32)
            pool_rd = dram.tile([P, F3], dt.float32)
            pool_ptB = dram.tile([P, F3], dt.float32)
            pool_rdB = dram.tile([P, F3], dt.float32)

            # two half-shard pool accumulators so the first AllReduce can be
            # emitted (and execute) while layer 3's second half still runs
            pool_accA = pool_ps.tile([P, F3], dt.float32, tag="plA")
            pool_accB = pool_ps.tile([P, F3], dt.float32, tag="plB")
            callno = [0]

            def layer(tbl, F_in, F_out, waug, brow, self_src, shard_out,
                      ag=None, pre_ag=None):
                last = F_in == F2  # layer 3
                for chn in range(NCH):
                    pac = pa_ps.tile([FMAX, CW * P], dt.float32, tag="pa",
                                     name="pa")
                    pas = [pac[:F_in, wl * P:(wl + 1) * P] for wl in range(CW)]
                    for g in range(NG):
                        if pre_ag is not None and chn == 0 and g == NG // 2:
                            # groups 0/1 read table segment 0 only; emit the
                            # previous layer's seg-1 AllGather here so the Pool
                            # stream keeps issuing gathers while it lands
                            pre_ag()
                        call0 = (chn * NG + g) * CALL_COLS
                        fat = fatp.tile([P, CALL_COLS, P], dt.bfloat16,
                                        tag="fat", name="fat")
                        nc.gpsimd.dma_gather(
                            out_ap=fat[:],
                            in_ap=tbl[g * GRP:(g + 1) * GRP, :],
                            idxs_ap=esrc16[:, call0 * 8:(call0 + CALL_COLS) * 8],
                            num_idxs=CALL_IDX,
                            num_idxs_reg=CALL_IDX,
                            elem_size=P,
                            single_packet=False,
                            queue_num=callno[0] % NQ,
                        )
                        callno[0] += 1
                        cs = slice(call0, call0 + CALL_COLS)
                        mw = mwp.tile([P, CALL_COLS, FMAX], dt.bfloat16,
                                      tag="mw", name="mw")[:, :, :F_in]
                        nc.vector.tensor_tensor(
                            out=mw[:], in0=fat[:, :, :F_in],
                            in1=enrm[:, cs, None].broadcast_to([P, CALL_COLS, F_in]),
                            op=OP.mult)
                        S = sp.tile([P, CALL_COLS, P], dt.bfloat16, tag="S", name="S")
                        nc.vector.tensor_tensor(
                            out=S[:],
                            in0=edl[:, cs, None].broadcast_to([P, CALL_COLS, P]),
                            in1=iota_b[:, None, :].broadcast_to([P, CALL_COLS, P]),
                            op=OP.is_equal)
                        for wloc in range(CW):
                            for t in range(TG):
                                c = wloc * TG + t
                                nc.tensor.matmul(
                                    out=pas[wloc][:], lhsT=mw[:, c, :],
                                    rhs=S[:, c, :],
                                    start=(g == 0 and t == 0), stop=False)
                    for wloc in range(CW):
                        w = chn * CW + wloc
                        # self-loop term: pa[f,d] += h[d,f] * dis2[d] via a
                        # matmul with a diagonal rhs (local shard rows)
                        hw_t = work.tile([P, FMAX], dt.bfloat16, tag="hw",
                                         name="hw_t")[:, :F_in]
                        nc.sync.dma_start(out=hw_t[:],
                                          in_=self_src[w * P:(w + 1) * P, :F_in])
                        Dd = sp.tile([P, P], dt.bfloat16, tag="Dd", name="Dd")
                        nc.vector.tensor_tensor(
                            out=Dd[:], in0=ident[:],
                            in1=dis2[:, w:w + 1].broadcast_to([P, P]), op=OP.mult)
                        nc.tensor.matmul(out=pas[wloc][:], lhsT=hw_t[:], rhs=Dd[:],
                                         start=False, stop=True)
                        aggT = work.tile([FMAX, P], dt.bfloat16, tag="aggT",
                                         name="aggT")[:F_in]
                        pa = pas[wloc]
                        nc.scalar.copy(out=aggT[:], in_=pa[:])
                        p2 = p2_ps.tile([P, F3], dt.float32, tag="p2",
                                        name="p2")[:, :F_out]
                        nc.tensor.matmul(out=p2[:], lhsT=aggT[:], rhs=waug[:F_in, :],
                                         start=True, stop=False)
                        nc.tensor.matmul(out=p2[:], lhsT=ones1[:], rhs=brow[:],
                                         start=False, stop=True)
                        h = work.tile([P, F3], dt.bfloat16, tag="h",
                                      name="h")[:, :F_out]
                        nc.scalar.activation(h[:], p2[:], AF.Relu)
                        if not last:
                            nc.sync.dma_start(
                                out=shard_out[w * P:(w + 1) * P, :F_out], in_=h[:])
                        else:
                            Sg = sp.tile([P, P], dt.bfloat16, tag="Sg", name="Sg")
                            nc.vector.tensor_tensor(
                                out=Sg[:],
                                in0=gloc[:, w:w + 1].broadcast_to([P, P]),
                                in1=iota_b[:], op=OP.is_equal)
                            pacc = pool_accA if w < NW // 2 else pool_accB
                            nc.tensor.matmul(out=pacc[:], lhsT=Sg[:], rhs=h[:],
                                             start=(w in (0, NW // 2)),
                                             stop=(w in (NW // 2 - 1, NW - 1)))
                    if ag is not None and chn == 9:
                        # emitted well after the seg-0 windows (chunks 0-6) so
                        # the in-order Pool stream reaches it with the data
                        # dependency already satisfied (no Q7 stall)
                        ag(0)
                    if last and chn == 12:
                        # flush + AllReduce the first pool half here: its data
                        # (windows 0-48) is long done, so the Pool stream does
                        # not stall and the collective overlaps chunks 12-13
                        psbA = work.tile([P, F3], dt.float32, tag="psb")
                        nc.scalar.copy(out=psbA[:], in_=pool_accA[:])
                        nc.sync.dma_start(out=pool_pt[:], in_=psbA[:])
                        nc.gpsimd.collective_compute(
                            "AllReduce", mybir.AluOpType.add, replica_groups=rg,
                            ins=[pool_pt.opt()], outs=[pool_rd.opt()])

            SEG = cfg.SHARD // 2
            HSEG = cfg.n_cores * SEG

            def seg_allgather(shard, table):
                def ag(sgi):
                    nc.gpsimd.collective_compute(
                        "AllGather", mybir.AluOpType.bypass, replica_groups=rg,
                        ins=[shard[sgi * SEG:(sgi + 1) * SEG, :].opt()],
                        outs=[table[sgi * HSEG:(sgi + 1) * HSEG, :].opt()])
                return ag

            ag1 = seg_allgather(h1s, h1t)
            ag2 = seg_allgather(h2s, h2t)
            layer(xt_d, F0, F1, w1a, b1r, xs_d, h1s, ag=ag1)
            layer(h1t, F1, F2, w2a, b2r, h1s, h2s, ag=ag2,
                  pre_ag=lambda: ag1(1))
            layer(h2t, F2, F3, w3a, b3r, h2s, None, pre_ag=lambda: ag2(1))

            # ---- pooling second half -> AllReduce -> combine -> mean
            psb = work.tile([P, F3], dt.float32, tag="psb")
            nc.scalar.copy(out=psb[:], in_=pool_accB[:])
            nc.sync.dma_start(out=pool_ptB[:], in_=psb[:])
            nc.gpsimd.collective_compute(
                "AllReduce", mybir.AluOpType.add, replica_groups=rg,
                ins=[pool_ptB.opt()], outs=[pool_rdB.opt()])
            poolr = work.tile([P, F3], dt.float32, tag="poolr")
            nc.sync.dma_start(out=poolr[:], in_=pool_rd[:])
            poolrB = work.tile([P, F3], dt.float32, tag="poolrB")
            nc.sync.dma_start(out=poolrB[:], in_=pool_rdB[:])
            nc.vector.tensor_tensor(out=poolr[:], in0=poolr[:], in1=poolrB[:],
                                    op=OP.add)
            pooled = work.tile([P, F3], dt.bfloat16, tag="pooled")
            nc.scalar.activation(pooled[:], poolr[:], AF.Copy, scale=invc[:])

            # ---- head: z1 = relu(pooled @ fW1 + fb1); z2 = z1 @ fW2 + fb2
            ptA_ps = head_ps.tile([F3 // 2, P], dt.bfloat16, tag="pt")
            nc.tensor.transpose(out=ptA_ps[:], in_=pooled[:, :F3 // 2], identity=ident[:])
            ptA = work.tile([F3 // 2, P], dt.bfloat16, tag="ptA")
            nc.scalar.copy(out=ptA[:], in_=ptA_ps[:])
            ptB_ps = head_ps.tile([F3 // 2, P], dt.bfloat16, tag="pt")
            nc.tensor.transpose(out=ptB_ps[:], in_=pooled[:, F3 // 2:], identity=ident[:])
            ptB = work.tile([F3 // 2, P], dt.bfloat16, tag="ptB")
            nc.scalar.copy(out=ptB[:], in_=ptB_ps[:])

            z1_ps = head_ps.tile([cfg.HID, P], dt.float32, tag="z1")
            nc.tensor.matmul(out=z1_ps[:], lhsT=fw1a[:], rhs=ptA[:], start=True, stop=False)
            nc.tensor.matmul(out=z1_ps[:], lhsT=fw1b[:], rhs=ptB[:], start=False, stop=True)
            z1 = work.tile([cfg.HID, P], dt.bfloat16, tag="z1s")
            nc.scalar.activation(z1[:], z1_ps[:], AF.Relu, bias=fb1c[:])

            z2_ps = head_ps.tile([1, P], dt.float32, tag="z2")
            nc.tensor.matmul(out=z2_ps[:], lhsT=fw2[:], rhs=z1[:], start=True, stop=True)
            z2 = work.tile([1, P], dt.float32, tag="z2s")
            nc.scalar.activation(z2[:], z2_ps[:], AF.Copy, bias=float(fb2))
            # softmax over a width-1 axis == 1.0 for finite logits
            outs = work.tile([1, P], dt.float32, tag="outs")
            nc.vector.tensor_tensor(out=outs[:], in0=z2[:], in1=z2[:], op=OP.is_equal)
            nc.sync.dma_start(out=out_d[:], in_=outs[:])

    nc.compile()
    return nc


# ---------------------------------------------------------------- run

_CACHE = {}


def _get_nc(cfg, fb2):
    key = (tuple(cfg.F), cfg.NW, cfg.TG, cfg.SHARD, fb2)
    if key not in _CACHE:
        _CACHE[key] = build_bass(cfg, fb2)
    return _CACHE[key]


def make_in_maps(cfg, inp):
    cores, xt, wts, fb2 = build_host_data(cfg, inp)
    in_maps = []
    for c in range(cfg.n_cores):
        m = dict(xt=xt, **cores[c], **wts)
        in_maps.append(m)
    return in_maps, fb2


def derive_cfg(inputs):
    """Auto-size TG to the densest (dst window, src group) cell."""
    cfg = CFG
    srcA = np.asarray(inputs["edge_index"][0]).astype(np.int64).ravel()
    dstA = np.asarray(inputs["edge_index"][1]).astype(np.int64).ravel()
    SEG = cfg.SHARD // 2
    vv = np.arange(cfg.NPAD, dtype=np.int64)
    vc, vr = vv // cfg.SHARD, vv % cfg.SHARD
    vs = vr // SEG
    remap = vs * (cfg.n_cores * SEG) + vc * SEG + (vr - vs * SEG)
    cell = (dstA // P) * cfg.NG + remap[srcA] // cfg.GRP
    ccnt = np.bincount(cell, minlength=(cfg.NPAD // P) * cfg.NG)
    need = max(1, int(-(-ccnt.max() // P)))
    if need != cfg.TG:
        cfg = GCNConfig(**{**cfg.__dict__, "TG": need})
    return cfg


def kernel(**inputs):
    cfg = derive_cfg(inputs)
    in_maps, fb2 = make_in_maps(cfg, inputs)
    nc = _get_nc(cfg, fb2)
    from concourse.bass_utils import run_bass_kernel_spmd
    res = run_bass_kernel_spmd(nc, in_maps, core_ids=list(range(cfg.n_cores)))
    out = np.asarray(res.results[0]["out"]).reshape(P)[:cfg.G]
    return out.reshape(cfg.G, 1).astype(np.float32)


# revision 32
# speedup vs baseline: 1.0301x; 1.0301x over previous
"""GCN (3-layer GCNConv + mean-pool + MLP head) Trainium2 Bass kernel, 8 NeuronCores.

Strategy (graph/data parallel, per sharding hint):
  - Destination nodes are partitioned into 8 contiguous blocks (one per core).
  - Host partitions the edge list (self-loops included as ordinary edges with
    norm 1/deg) by destination block, then by destination window (128 dst
    nodes per window) and by source-range group (4 groups of 25088 table rows,
    so gather indices fit int16), laying edges out on a [128, COLS] grid.
  - Per layer, each core bulk-gathers source rows with InstDMAGatherAnt
    (dma_gather): one call per (chunk of 7 windows x source group), thousands
    of rows per call, from a 256B-row padded node table in DRAM. This is
    ~2x cheaper per row in GPSIMD descriptor-generation time than per-window
    indirect DMAs and amortizes instruction overhead ~40x.
  - Messages are scaled by the GCN edge norm (one vector op per call) and
    segment-summed into destination windows with one-hot x message matmuls
    accumulated in PSUM (aggregate-then-transform).
  - Per-shard layer output (relu(agg @ W + b)) is written into a padded
    [SHARD, 128] bounce buffer and AllGathered into the next layer's table.
  - Layer 3 output is mean-pooled per graph locally (one-hot matmul into a
    PSUM accumulator), AllReduced across cores, and the tiny FC head runs
    replicated on every core.
"""

import os
import sys
from dataclasses import dataclass

import numpy as np
import ml_dtypes

for _p in ("/opt/trn_rl_repo", "/root/.axon_site/_ro/trn_rl_repo"):
    if os.path.isdir(_p) and _p not in sys.path:
        sys.path.insert(0, _p)

bf16 = ml_dtypes.bfloat16
P = 128


@dataclass
class GCNConfig:
    N: int = 100000          # real nodes
    G: int = 128             # graphs (output width; PSUM col budget)
    SHARD: int = 12544       # padded nodes per core (NW * 128)
    NW: int = 98             # dst windows per core
    NG: int = 4              # source-range groups (int16 gather index limit)
    TG: int = 5              # edge columns per (window, group); auto-derived
    CW: int = 7              # windows per gather chunk
    F: tuple = (40, 40, 80, 160)   # feature dims x -> h1 -> h2 -> h3
    HID: int = 128           # fc hidden
    n_cores: int = 8

    @property
    def NPAD(self):
        return self.n_cores * self.SHARD

    @property
    def GRP(self):
        return self.NPAD // self.NG

    @property
    def COLS(self):
        # column c of the grid: call (ch, g) covers CW*TG contiguous columns;
        # global col = ((ch*NG + g)*CW + wloc)*TG + t
        return self.NW * self.NG * self.TG

    @property
    def NCH(self):
        return self.NW // self.CW


CFG = GCNConfig()


# ---------------------------------------------------------------- host prep

def build_host_data(cfg, inp):
    """Partition/sort edges (incl self-loops) by (dst window, src group),
    compute GCN edge norms, build per-core gather-index + norm/slot grids."""
    N, SHARD, NW, NG, TG, CW = cfg.N, cfg.SHARD, cfg.NW, cfg.NG, cfg.TG, cfg.CW
    GRP = cfg.GRP
    src = np.asarray(inp["edge_index"][0]).astype(np.int64).ravel()
    dst = np.asarray(inp["edge_index"][1]).astype(np.int64).ravel()
    batch = np.asarray(inp["batch"]).astype(np.int64).ravel()
    deg = (np.bincount(dst, minlength=N) + 1).astype(np.float32)
    dis = 1.0 / np.sqrt(deg)
    # self-loops (weight 1/deg) are applied via a dense per-window diagonal
    # term on the local shard, not via the gather grid
    srcA, dstA = src, dst
    norm = (dis[src] * dis[dst]).astype(np.float32)

    # segment-major table layout: two AllGather segments per layer boundary.
    SEG = SHARD // 2
    vv = np.arange(cfg.NPAD, dtype=np.int64)
    vc, vr = vv // SHARD, vv % SHARD
    vs = vr // SEG
    remap = vs * (cfg.n_cores * SEG) + vc * SEG + (vr - vs * SEG)

    srcR = remap[srcA]               # remapped source row in the padded table
    grp = srcR // GRP                # source group (gather call bucket)
    core = dstA // SHARD
    win = (dstA % SHARD) // P
    dloc = (dstA % SHARD) % P
    order = np.lexsort((grp, win, core))
    srcR, grp, core, win, dloc, norm = (
        a[order] for a in (srcR, grp, core, win, dloc, norm))

    cores = []
    for c in range(cfg.n_cores):
        m = core == c
        sr, gr, wn, dl, nm = srcR[m], grp[m], win[m], dloc[m], norm[m]
        esrc16 = np.zeros((16, cfg.COLS * P // 16), np.int16)
        edl = np.full((P, cfg.COLS), -1.0, bf16)
        enrm = np.zeros((P, cfg.COLS), bf16)
        # edges sorted by (win, grp); per (win, grp) cell: slot j ->
        # global col = ((ch*NG + g)*CW + wloc)*TG + j//128, p = j%128
        cell = wn * NG + gr                      # 0 .. NW*NG-1
        ccnt = np.bincount(cell, minlength=NW * NG)
        assert ccnt.max() <= TG * P, f"cell overflow: {ccnt.max()} > {TG * P}"
        jin = (np.concatenate([np.arange(n) for n in ccnt])
               if len(cell) else np.zeros(0, np.int64))
        ch = wn // CW
        wloc = wn % CW
        col = ((ch * NG + gr) * CW + wloc) * TG + jin // P
        pp = jin % P
        ipos = col * P + pp                      # dma_gather index position
        esrc16[ipos % 16, ipos // 16] = (sr - gr * GRP).astype(np.int16)
        edl[pp, col] = dl.astype(bf16)
        enrm[pp, col] = nm.astype(bf16)
        nid = np.arange(SHARD) + c * SHARD
        gl = np.where(nid < N, batch[np.minimum(nid, N - 1)], -1).astype(np.float32)
        gloc = np.ascontiguousarray(gl.reshape(NW, P).T).astype(bf16)
        d2 = np.where(nid < N, 1.0 / deg[np.minimum(nid, N - 1)], 0.0).astype(np.float32)
        dis2 = np.ascontiguousarray(d2.reshape(NW, P).T).astype(bf16)
        xs = np.zeros((SHARD, cfg.F[0]), bf16)
        nreal = max(0, min(SHARD, N - c * SHARD))
        xs[:nreal] = np.asarray(inp["x"])[c * SHARD:c * SHARD + nreal].astype(bf16)
        cores.append(dict(esrc16=np.tile(esrc16, (8, 1)), edl=edl, enrm=enrm,
                          gloc=gloc, dis2=dis2, xs=xs))

    xt = np.zeros((cfg.NPAD, P), bf16)
    xt[remap[:N], :cfg.F[0]] = np.asarray(inp["x"]).astype(bf16)

    cnt = np.bincount(batch, minlength=cfg.G).astype(np.float32)
    invc = np.zeros((P, 1), np.float32)
    invc[:cfg.G, 0] = 1.0 / np.maximum(cnt, 1.0)

    def a2(x, dt):
        return np.ascontiguousarray(np.asarray(x), dtype=dt)

    wts = dict(
        w1a=np.concatenate([a2(inp["W1"], bf16), a2(inp["b1"], bf16)[None]], 0),
        w2a=np.concatenate([a2(inp["W2"], bf16), a2(inp["b2"], bf16)[None]], 0),
        w3a=np.concatenate([a2(inp["W3"], bf16), a2(inp["b3"], bf16)[None]], 0),
        fw1=a2(inp["fW1"], bf16),
        fb1c=a2(inp["fb1"], np.float32).reshape(-1, 1),
        fw2=a2(inp["fW2"], bf16),
        invc=invc,
    )
    fb2 = float(np.asarray(inp["fb2"]).ravel()[0])
    return cores, xt, wts, fb2


# ---------------------------------------------------------------- bass build

def build_bass(cfg, fb2):
    import concourse.bacc as bacc
    import concourse.bass as bass
    import concourse.mybir as mybir
    import concourse.tile as tile
    from concourse.masks import make_identity

    dt = mybir.dt
    AF = mybir.ActivationFunctionType
    OP = mybir.AluOpType
    F0, F1, F2, F3 = cfg.F
    NW, NG, TG, CW, NCH, G = cfg.NW, cfg.NG, cfg.TG, cfg.CW, cfg.NCH, cfg.G
    GRP = cfg.GRP
    FMAX = max(F0, F1, F2)
    CALL_COLS = CW * TG
    CALL_IDX = CALL_COLS * P
    NQ = 4

    nc = bacc.Bacc("TRN2", target_bir_lowering=False, debug=False,
                   enable_asserts=False, num_devices=cfg.n_cores,
                   num_swdge_queues=NQ)

    # ---- I/O
    xt_d = nc.dram_tensor("xt", [cfg.NPAD, P], dt.bfloat16, kind="ExternalInput")
    esrc_d = nc.dram_tensor("esrc16", [P, cfg.COLS * P // 16], dt.int16,
                            kind="ExternalInput")
    edl_d = nc.dram_tensor("edl", [P, cfg.COLS], dt.bfloat16, kind="ExternalInput")
    enrm_d = nc.dram_tensor("enrm", [P, cfg.COLS], dt.bfloat16, kind="ExternalInput")
    gloc_d = nc.dram_tensor("gloc", [P, NW], dt.bfloat16, kind="ExternalInput")
    dis2_d = nc.dram_tensor("dis2", [P, NW], dt.bfloat16, kind="ExternalInput")
    xs_d = nc.dram_tensor("xs", [cfg.SHARD, F0], dt.bfloat16, kind="ExternalInput")
    w1a_d = nc.dram_tensor("w1a", [F0 + 1, F1], dt.bfloat16, kind="ExternalInput")
    w2a_d = nc.dram_tensor("w2a", [F1 + 1, F2], dt.bfloat16, kind="ExternalInput")
    w3a_d = nc.dram_tensor("w3a", [F2 + 1, F3], dt.bfloat16, kind="ExternalInput")
    fw1_d = nc.dram_tensor("fw1", [F3, cfg.HID], dt.bfloat16, kind="ExternalInput")
    fb1_d = nc.dram_tensor("fb1c", [cfg.HID, 1], dt.float32, kind="ExternalInput")
    fw2_d = nc.dram_tensor("fw2", [cfg.HID, 1], dt.bfloat16, kind="ExternalInput")
    invc_d = nc.dram_tensor("invc", [P, 1], dt.float32, kind="ExternalInput")
    out_d = nc.dram_tensor("out", [1, P], dt.float32, kind="ExternalOutput")

    rg = [list(range(cfg.n_cores))]

    with tile.TileContext(nc) as tc:
        with (
            tc.tile_pool(name="res", bufs=1) as res,                  # persistent SBUF
            tc.tile_pool(name="fat", bufs=8) as fatp,
            tc.tile_pool(name="mw", bufs=5) as mwp,
            tc.tile_pool(name="sp", bufs=6) as sp,
            tc.tile_pool(name="work", bufs=2) as work,
            tc.tile_pool(name="pa_ps", bufs=1, space="PSUM") as pa_ps,
            tc.tile_pool(name="p2_ps", bufs=2, space="PSUM") as p2_ps,
            tc.tile_pool(name="pool_ps", bufs=1, space="PSUM") as pool_ps,
            tc.tile_pool(name="head_ps", bufs=1, space="PSUM") as head_ps,
            tc.tile_pool(name="dram", bufs=1, space="DRAM") as dram,
        ):
            # ---- load persistent SBUF state
            esrc16 = res.tile([P, cfg.COLS * P // 16], dt.int16)
            edl = res.tile([P, cfg.COLS], dt.bfloat16)
            enrm = res.tile([P, cfg.COLS], dt.bfloat16)
            gloc = res.tile([P, NW], dt.bfloat16)
            dis2 = res.tile([P, NW], dt.bfloat16)
            w1a = res.tile([F0 + 1, F1], dt.bfloat16)
            w2a = res.tile([F1 + 1, F2], dt.bfloat16)
            w3a = res.tile([F2 + 1, F3], dt.bfloat16)
            fw1a = res.tile([F3 // 2, cfg.HID], dt.bfloat16)
            fw1b = res.tile([F3 // 2, cfg.HID], dt.bfloat16)
            fb1c = res.tile([cfg.HID, 1], dt.float32)
            fw2 = res.tile([cfg.HID, 1], dt.bfloat16)
            invc = res.tile([P, 1], dt.float32)
            b1r = res.tile([1, F1], dt.bfloat16)
            b2r = res.tile([1, F2], dt.bfloat16)
            b3r = res.tile([1, F3], dt.bfloat16)
            for sb, dr in ((esrc16, esrc_d), (edl, edl_d), (enrm, enrm_d),
                           (gloc, gloc_d), (dis2, dis2_d), (w1a, w1a_d), (w2a, w2a_d),
                           (w3a, w3a_d), (fb1c, fb1_d), (fw2, fw2_d),
                           (invc, invc_d)):
                nc.sync.dma_start(out=sb[:], in_=dr[:])
            nc.sync.dma_start(out=b1r[:], in_=w1a_d[F0:F0 + 1, :])
            nc.sync.dma_start(out=b2r[:], in_=w2a_d[F1:F1 + 1, :])
            nc.sync.dma_start(out=b3r[:], in_=w3a_d[F2:F2 + 1, :])
            nc.sync.dma_start(out=fw1a[:], in_=fw1_d[0:F3 // 2, :])
            nc.sync.dma_start(out=fw1b[:], in_=fw1_d[F3 // 2:, :])

            iota_i = res.tile([P, P], dt.int32)
            nc.gpsimd.iota(iota_i[:], pattern=[[1, P]], base=0, channel_multiplier=0)
            iota_b = res.tile([P, P], dt.bfloat16)
            nc.vector.tensor_copy(out=iota_b[:], in_=iota_i[:])
            ident = res.tile([P, P], dt.bfloat16)
            make_identity(nc, ident[:])
            ones1 = res.tile([1, P], dt.bfloat16)
            nc.vector.memset(ones1[:], 1.0)

            # ---- DRAM tables / bounce buffers (256B-row padded)
            h1s = dram.tile([cfg.SHARD, P], dt.bfloat16)
            h2s = dram.tile([cfg.SHARD, P], dt.bfloat16)
            h1t = dram.tile([cfg.NPAD, P], dt.bfloat16)
            h2t = dram.tile([cfg.NPAD, P], dt.bfloat16)
            pool_pt = dram.tile([P, F3], dt.float32)
            pool_rd = dram.tile([P, F3], dt.float32)

            pool_acc = pool_ps.tile([P, F3], dt.float32)
            callno = [0]

            def layer(tbl, F_in, F_out, waug, brow, self_src, shard_out,
                      ag=None, pre_ag=None):
                last = F_in == F2  # layer 3
                for chn in range(NCH):
                    pac = pa_ps.tile([FMAX, CW * P], dt.float32, tag="pa",
                                     name="pa")
                    pas = [pac[:F_in, wl * P:(wl + 1) * P] for wl in range(CW)]
                    for g in range(NG):
                        if pre_ag is not None and chn == 0 and g == NG // 2:
                            # groups 0/1 read table segment 0 only; emit the
                            # previous layer's seg-1 AllGather here so the Pool
                            # stream keeps issuing gathers while it lands
                            pre_ag()
                        call0 = (chn * NG + g) * CALL_COLS
                        fat = fatp.tile([P, CALL_COLS, P], dt.bfloat16,
                                        tag="fat", name="fat")
                        nc.gpsimd.dma_gather(
                            out_ap=fat[:],
                            in_ap=tbl[g * GRP:(g + 1) * GRP, :],
                            idxs_ap=esrc16[:, call0 * 8:(call0 + CALL_COLS) * 8],
                            num_idxs=CALL_IDX,
                            num_idxs_reg=CALL_IDX,
                            elem_size=P,
                            single_packet=False,
                            queue_num=callno[0] % NQ,
                        )
                        callno[0] += 1
                        cs = slice(call0, call0 + CALL_COLS)
                        mw = mwp.tile([P, CALL_COLS, FMAX], dt.bfloat16,
                                      tag="mw", name="mw")[:, :, :F_in]
                        nc.vector.tensor_tensor(
                            out=mw[:], in0=fat[:, :, :F_in],
                            in1=enrm[:, cs, None].broadcast_to([P, CALL_COLS, F_in]),
                            op=OP.mult)
                        S = sp.tile([P, CALL_COLS, P], dt.bfloat16, tag="S", name="S")
                        nc.vector.tensor_tensor(
                            out=S[:],
                            in0=edl[:, cs, None].broadcast_to([P, CALL_COLS, P]),
                            in1=iota_b[:, None, :].broadcast_to([P, CALL_COLS, P]),
                            op=OP.is_equal)
                        for wloc in range(CW):
                            for t in range(TG):
                                c = wloc * TG + t
                                nc.tensor.matmul(
                                    out=pas[wloc][:], lhsT=mw[:, c, :],
                                    rhs=S[:, c, :],
                                    start=(g == 0 and t == 0), stop=False)
                    for wloc in range(CW):
                        w = chn * CW + wloc
                        # self-loop term: pa[f,d] += h[d,f] * dis2[d] via a
                        # matmul with a diagonal rhs (local shard rows)
                        hw_t = work.tile([P, FMAX], dt.bfloat16, tag="hw",
                                         name="hw_t")[:, :F_in]
                        nc.sync.dma_start(out=hw_t[:],
                                          in_=self_src[w * P:(w + 1) * P, :F_in])
                        Dd = sp.tile([P, P], dt.bfloat16, tag="Dd", name="Dd")
                        nc.vector.tensor_tensor(
                            out=Dd[:], in0=ident[:],
                            in1=dis2[:, w:w + 1].broadcast_to([P, P]), op=OP.mult)
                        nc.tensor.matmul(out=pas[wloc][:], lhsT=hw_t[:], rhs=Dd[:],
                                         start=False, stop=True)
                        aggT = work.tile([FMAX, P], dt.bfloat16, tag="aggT",
                                         name="aggT")[:F_in]
                        pa = pas[wloc]
                        nc.scalar.copy(out=aggT[:], in_=pa[:])
                        p2 = p2_ps.tile([P, F3], dt.float32, tag="p2",
                                        name="p2")[:, :F_out]
                        nc.tensor.matmul(out=p2[:], lhsT=aggT[:], rhs=waug[:F_in, :],
                                         start=True, stop=False)
                        nc.tensor.matmul(out=p2[:], lhsT=ones1[:], rhs=brow[:],
                                         start=False, stop=True)
                        h = work.tile([P, F3], dt.bfloat16, tag="h",
                                      name="h")[:, :F_out]
                        nc.scalar.activation(h[:], p2[:], AF.Relu)
                        if not last:
                            nc.sync.dma_start(
                                out=shard_out[w * P:(w + 1) * P, :F_out], in_=h[:])
                        else:
                            Sg = sp.tile([P, P], dt.bfloat16, tag="Sg", name="Sg")
                            nc.vector.tensor_tensor(
                                out=Sg[:],
                                in0=gloc[:, w:w + 1].broadcast_to([P, P]),
                                in1=iota_b[:], op=OP.is_equal)
                            nc.tensor.matmul(out=pool_acc[:], lhsT=Sg[:], rhs=h[:],
                                             start=(w == 0), stop=(w == NW - 1))
                    if ag is not None and chn == 9:
                        # emitted well after the seg-0 windows (chunks 0-6) so
                        # the in-order Pool stream reaches it with the data
                        # dependency already satisfied (no Q7 stall)
                        ag(0)

            SEG = cfg.SHARD // 2
            HSEG = cfg.n_cores * SEG

            def seg_allgather(shard, table):
                def ag(sgi):
                    nc.gpsimd.collective_compute(
                        "AllGather", mybir.AluOpType.bypass, replica_groups=rg,
                        ins=[shard[sgi * SEG:(sgi + 1) * SEG, :].opt()],
                        outs=[table[sgi * HSEG:(sgi + 1) * HSEG, :].opt()])
                return ag

            ag1 = seg_allgather(h1s, h1t)
            ag2 = seg_allgather(h2s, h2t)
            layer(xt_d, F0, F1, w1a, b1r, xs_d, h1s, ag=ag1)
            layer(h1t, F1, F2, w2a, b2r, h1s, h2s, ag=ag2,
                  pre_ag=lambda: ag1(1))
            layer(h2t, F2, F3, w3a, b3r, h2s, None, pre_ag=lambda: ag2(1))

            # ---- pooling partial -> AllReduce -> mean
            psb = work.tile([P, F3], dt.float32, tag="psb")
            nc.scalar.copy(out=psb[:], in_=pool_acc[:])
            nc.sync.dma_start(out=pool_pt[:], in_=psb[:])
            nc.gpsimd.collective_compute(
                "AllReduce", mybir.AluOpType.add, replica_groups=rg,
                ins=[pool_pt.opt()], outs=[pool_rd.opt()])
            poolr = work.tile([P, F3], dt.float32, tag="poolr")
            nc.sync.dma_start(out=poolr[:], in_=pool_rd[:])
            pooled = work.tile([P, F3], dt.bfloat16, tag="pooled")
            nc.scalar.activation(pooled[:], poolr[:], AF.Copy, scale=invc[:])

            # ---- head: z1 = relu(pooled @ fW1 + fb1); z2 = z1 @ fW2 + fb2
            ptA_ps = head_ps.tile([F3 // 2, P], dt.bfloat16, tag="pt")
            nc.tensor.transpose(out=ptA_ps[:], in_=pooled[:, :F3 // 2], identity=ident[:])
            ptA = work.tile([F3 // 2, P], dt.bfloat16, tag="ptA")
            nc.scalar.copy(out=ptA[:], in_=ptA_ps[:])
            ptB_ps = head_ps.tile([F3 // 2, P], dt.bfloat16, tag="pt")
            nc.tensor.transpose(out=ptB_ps[:], in_=pooled[:, F3 // 2:], identity=ident[:])
            ptB = work.tile([F3 // 2, P], dt.bfloat16, tag="ptB")
            nc.scalar.copy(out=ptB[:], in_=ptB_ps[:])

            z1_ps = head_ps.tile([cfg.HID, P], dt.float32, tag="z1")
            nc.tensor.matmul(out=z1_ps[:], lhsT=fw1a[:], rhs=ptA[:], start=True, stop=False)
            nc.tensor.matmul(out=z1_ps[:], lhsT=fw1b[:], rhs=ptB[:], start=False, stop=True)
            z1 = work.tile([cfg.HID, P], dt.bfloat16, tag="z1s")
            nc.scalar.activation(z1[:], z1_ps[:], AF.Relu, bias=fb1c[:])

            z2_ps = head_ps.tile([1, P], dt.float32, tag="z2")
            nc.tensor.matmul(out=z2_ps[:], lhsT=fw2[:], rhs=z1[:], start=True, stop=True)
            z2 = work.tile([1, P], dt.float32, tag="z2s")
            nc.scalar.activation(z2[:], z2_ps[:], AF.Copy, bias=float(fb2))
            # softmax over a width-1 axis == 1.0 for finite logits
            outs = work.tile([1, P], dt.float32, tag="outs")
            nc.vector.tensor_tensor(out=outs[:], in0=z2[:], in1=z2[:], op=OP.is_equal)
            nc.sync.dma_start(out=out_d[:], in_=outs[:])

    nc.compile()
    return nc


# ---------------------------------------------------------------- run

_CACHE = {}


def _get_nc(cfg, fb2):
    key = (tuple(cfg.F), cfg.NW, cfg.TG, cfg.SHARD, fb2)
    if key not in _CACHE:
        _CACHE[key] = build_bass(cfg, fb2)
    return _CACHE[key]


def make_in_maps(cfg, inp):
    cores, xt, wts, fb2 = build_host_data(cfg, inp)
    in_maps = []
    for c in range(cfg.n_cores):
        m = dict(xt=xt, **cores[c], **wts)
        in_maps.append(m)
    return in_maps, fb2


def derive_cfg(inputs):
    """Auto-size TG to the densest (dst window, src group) cell."""
    cfg = CFG
    srcA = np.asarray(inputs["edge_index"][0]).astype(np.int64).ravel()
    dstA = np.asarray(inputs["edge_index"][1]).astype(np.int64).ravel()
    SEG = cfg.SHARD // 2
    vv = np.arange(cfg.NPAD, dtype=np.int64)
    vc, vr = vv // cfg.SHARD, vv % cfg.SHARD
    vs = vr // SEG
    remap = vs * (cfg.n_cores * SEG) + vc * SEG + (vr - vs * SEG)
    cell = (dstA // P) * cfg.NG + remap[srcA] // cfg.GRP
    ccnt = np.bincount(cell, minlength=(cfg.NPAD // P) * cfg.NG)
    need = max(1, int(-(-ccnt.max() // P)))
    if need != cfg.TG:
        cfg = GCNConfig(**{**cfg.__dict__, "TG": need})
    return cfg


def kernel(**inputs):
    cfg = derive_cfg(inputs)
    in_maps, fb2 = make_in_maps(cfg, inputs)
    nc = _get_nc(cfg, fb2)
    from concourse.bass_utils import run_bass_kernel_spmd
    res = run_bass_kernel_spmd(nc, in_maps, core_ids=list(range(cfg.n_cores)))
    out = np.asarray(res.results[0]["out"]).reshape(P)[:cfg.G]
    return out.reshape(cfg.G, 1).astype(np.float32)


# revision 33
# speedup vs baseline: 1.0485x; 1.0178x over previous
"""GCN (3-layer GCNConv + mean-pool + MLP head) Trainium2 Bass kernel, 8 NeuronCores.

Strategy (graph/data parallel, per sharding hint):
  - Destination nodes are partitioned into 8 contiguous blocks (one per core).
  - Host partitions the edge list (self-loops included as ordinary edges with
    norm 1/deg) by destination block, then by destination window (128 dst
    nodes per window) and by source-range group (4 groups of 25088 table rows,
    so gather indices fit int16), laying edges out on a [128, COLS] grid.
  - Per layer, each core bulk-gathers source rows with InstDMAGatherAnt
    (dma_gather): one call per (chunk of 7 windows x source group), thousands
    of rows per call, from a 256B-row padded node table in DRAM. This is
    ~2x cheaper per row in GPSIMD descriptor-generation time than per-window
    indirect DMAs and amortizes instruction overhead ~40x.
  - Messages are scaled by the GCN edge norm (one vector op per call) and
    segment-summed into destination windows with one-hot x message matmuls
    accumulated in PSUM (aggregate-then-transform).
  - Per-shard layer output (relu(agg @ W + b)) is written into a padded
    [SHARD, 128] bounce buffer and AllGathered into the next layer's table.
  - Layer 3 output is mean-pooled per graph locally (one-hot matmul into a
    PSUM accumulator), AllReduced across cores, and the tiny FC head runs
    replicated on every core.
"""

import os
import sys
from dataclasses import dataclass

import numpy as np
import ml_dtypes

for _p in ("/opt/trn_rl_repo", "/root/.axon_site/_ro/trn_rl_repo"):
    if os.path.isdir(_p) and _p not in sys.path:
        sys.path.insert(0, _p)

bf16 = ml_dtypes.bfloat16
P = 128


@dataclass
class GCNConfig:
    N: int = 100000          # real nodes
    G: int = 128             # graphs (output width; PSUM col budget)
    SHARD: int = 12544       # padded nodes per core (NW * 128)
    NW: int = 98             # dst windows per core
    NG: int = 4              # source-range groups (int16 gather index limit)
    TG: int = 5              # edge columns per (window, group); auto-derived
    CW: int = 7              # windows per gather chunk
    F: tuple = (40, 40, 80, 160)   # feature dims x -> h1 -> h2 -> h3
    HID: int = 128           # fc hidden
    n_cores: int = 8

    @property
    def NPAD(self):
        return self.n_cores * self.SHARD

    @property
    def GRP(self):
        return self.NPAD // self.NG

    @property
    def COLS(self):
        # column c of the grid: call (ch, g) covers CW*TG contiguous columns;
        # global col = ((ch*NG + g)*CW + wloc)*TG + t
        return self.NW * self.NG * self.TG

    @property
    def NCH(self):
        return self.NW // self.CW


CFG = GCNConfig()


# ---------------------------------------------------------------- host prep

def build_host_data(cfg, inp):
    """Partition/sort edges (incl self-loops) by (dst window, src group),
    compute GCN edge norms, build per-core gather-index + norm/slot grids."""
    N, SHARD, NW, NG, TG, CW = cfg.N, cfg.SHARD, cfg.NW, cfg.NG, cfg.TG, cfg.CW
    GRP = cfg.GRP
    src = np.asarray(inp["edge_index"][0]).astype(np.int64).ravel()
    dst = np.asarray(inp["edge_index"][1]).astype(np.int64).ravel()
    batch = np.asarray(inp["batch"]).astype(np.int64).ravel()
    deg = (np.bincount(dst, minlength=N) + 1).astype(np.float32)
    dis = 1.0 / np.sqrt(deg)
    # self-loops (weight 1/deg) are applied via a dense per-window diagonal
    # term on the local shard, not via the gather grid
    srcA, dstA = src, dst
    norm = (dis[src] * dis[dst]).astype(np.float32)

    # segment-major table layout: two AllGather segments per layer boundary.
    SEG = SHARD // 2
    vv = np.arange(cfg.NPAD, dtype=np.int64)
    vc, vr = vv // SHARD, vv % SHARD
    vs = vr // SEG
    remap = vs * (cfg.n_cores * SEG) + vc * SEG + (vr - vs * SEG)

    srcR = remap[srcA]               # remapped source row in the padded table
    grp = srcR // GRP                # source group (gather call bucket)
    core = dstA // SHARD
    win = (dstA % SHARD) // P
    dloc = (dstA % SHARD) % P
    order = np.lexsort((grp, win, core))
    srcR, grp, core, win, dloc, norm = (
        a[order] for a in (srcR, grp, core, win, dloc, norm))

    cores = []
    for c in range(cfg.n_cores):
        m = core == c
        sr, gr, wn, dl, nm = srcR[m], grp[m], win[m], dloc[m], norm[m]
        esrc16 = np.zeros((16, cfg.COLS * P // 16), np.int16)
        edl = np.full((P, cfg.COLS), -1.0, bf16)
        enrm = np.zeros((P, cfg.COLS), bf16)
        # edges sorted by (win, grp); per (win, grp) cell: slot j ->
        # global col = ((ch*NG + g)*CW + wloc)*TG + j//128, p = j%128
        cell = wn * NG + gr                      # 0 .. NW*NG-1
        ccnt = np.bincount(cell, minlength=NW * NG)
        assert ccnt.max() <= TG * P, f"cell overflow: {ccnt.max()} > {TG * P}"
        jin = (np.concatenate([np.arange(n) for n in ccnt])
               if len(cell) else np.zeros(0, np.int64))
        ch = wn // CW
        wloc = wn % CW
        col = ((ch * NG + gr) * CW + wloc) * TG + jin // P
        pp = jin % P
        ipos = col * P + pp                      # dma_gather index position
        esrc16[ipos % 16, ipos // 16] = (sr - gr * GRP).astype(np.int16)
        edl[pp, col] = dl.astype(bf16)
        enrm[pp, col] = nm.astype(bf16)
        nid = np.arange(SHARD) + c * SHARD
        gl = np.where(nid < N, batch[np.minimum(nid, N - 1)], -1).astype(np.float32)
        gloc = np.ascontiguousarray(gl.reshape(NW, P).T).astype(bf16)
        d2 = np.where(nid < N, 1.0 / deg[np.minimum(nid, N - 1)], 0.0).astype(np.float32)
        dis2 = np.ascontiguousarray(d2.reshape(NW, P).T).astype(bf16)
        xs = np.zeros((SHARD, cfg.F[0]), bf16)
        nreal = max(0, min(SHARD, N - c * SHARD))
        xs[:nreal] = np.asarray(inp["x"])[c * SHARD:c * SHARD + nreal].astype(bf16)
        cores.append(dict(esrc16=np.tile(esrc16, (8, 1)), edl=edl, enrm=enrm,
                          gloc=gloc, dis2=dis2, xs=xs))

    xt = np.zeros((cfg.NPAD, P), bf16)
    xt[remap[:N], :cfg.F[0]] = np.asarray(inp["x"]).astype(bf16)

    cnt = np.bincount(batch, minlength=cfg.G).astype(np.float32)
    invc = np.zeros((P, 1), np.float32)
    invc[:cfg.G, 0] = 1.0 / np.maximum(cnt, 1.0)

    def a2(x, dt):
        return np.ascontiguousarray(np.asarray(x), dtype=dt)

    wts = dict(
        w1a=np.concatenate([a2(inp["W1"], bf16), a2(inp["b1"], bf16)[None]], 0),
        w2a=np.concatenate([a2(inp["W2"], bf16), a2(inp["b2"], bf16)[None]], 0),
        w3a=np.concatenate([a2(inp["W3"], bf16), a2(inp["b3"], bf16)[None]], 0),
        fw1=a2(inp["fW1"], bf16),
        fb1c=a2(inp["fb1"], np.float32).reshape(-1, 1),
        fw2=a2(inp["fW2"], bf16),
        invc=invc,
    )
    fb2 = float(np.asarray(inp["fb2"]).ravel()[0])
    return cores, xt, wts, fb2


# ---------------------------------------------------------------- bass build

def build_bass(cfg, fb2):
    import concourse.bacc as bacc
    import concourse.bass as bass
    import concourse.mybir as mybir
    import concourse.tile as tile
    from concourse.masks import make_identity

    dt = mybir.dt
    AF = mybir.ActivationFunctionType
    OP = mybir.AluOpType
    F0, F1, F2, F3 = cfg.F
    NW, NG, TG, CW, NCH, G = cfg.NW, cfg.NG, cfg.TG, cfg.CW, cfg.NCH, cfg.G
    GRP = cfg.GRP
    FMAX = max(F0, F1, F2)
    CALL_COLS = CW * TG
    CALL_IDX = CALL_COLS * P
    NQ = 4

    nc = bacc.Bacc("TRN2", target_bir_lowering=False, debug=False,
                   enable_asserts=False, num_devices=cfg.n_cores,
                   num_swdge_queues=NQ)

    # ---- I/O
    xt_d = nc.dram_tensor("xt", [cfg.NPAD, P], dt.bfloat16, kind="ExternalInput")
    esrc_d = nc.dram_tensor("esrc16", [P, cfg.COLS * P // 16], dt.int16,
                            kind="ExternalInput")
    edl_d = nc.dram_tensor("edl", [P, cfg.COLS], dt.bfloat16, kind="ExternalInput")
    enrm_d = nc.dram_tensor("enrm", [P, cfg.COLS], dt.bfloat16, kind="ExternalInput")
    gloc_d = nc.dram_tensor("gloc", [P, NW], dt.bfloat16, kind="ExternalInput")
    dis2_d = nc.dram_tensor("dis2", [P, NW], dt.bfloat16, kind="ExternalInput")
    xs_d = nc.dram_tensor("xs", [cfg.SHARD, F0], dt.bfloat16, kind="ExternalInput")
    w1a_d = nc.dram_tensor("w1a", [F0 + 1, F1], dt.bfloat16, kind="ExternalInput")
    w2a_d = nc.dram_tensor("w2a", [F1 + 1, F2], dt.bfloat16, kind="ExternalInput")
    w3a_d = nc.dram_tensor("w3a", [F2 + 1, F3], dt.bfloat16, kind="ExternalInput")
    fw1_d = nc.dram_tensor("fw1", [F3, cfg.HID], dt.bfloat16, kind="ExternalInput")
    fb1_d = nc.dram_tensor("fb1c", [cfg.HID, 1], dt.float32, kind="ExternalInput")
    fw2_d = nc.dram_tensor("fw2", [cfg.HID, 1], dt.bfloat16, kind="ExternalInput")
    invc_d = nc.dram_tensor("invc", [P, 1], dt.float32, kind="ExternalInput")
    out_d = nc.dram_tensor("out", [1, P], dt.float32, kind="ExternalOutput")

    rg = [list(range(cfg.n_cores))]

    with tile.TileContext(nc) as tc:
        with (
            tc.tile_pool(name="res", bufs=1) as res,                  # persistent SBUF
            tc.tile_pool(name="fat", bufs=9) as fatp,
            tc.tile_pool(name="mw", bufs=5) as mwp,
            tc.tile_pool(name="sp", bufs=5) as sp,
            tc.tile_pool(name="work", bufs=2) as work,
            tc.tile_pool(name="pa_ps", bufs=1, space="PSUM") as pa_ps,
            tc.tile_pool(name="p2_ps", bufs=2, space="PSUM") as p2_ps,
            tc.tile_pool(name="pool_ps", bufs=1, space="PSUM") as pool_ps,
            tc.tile_pool(name="head_ps", bufs=1, space="PSUM") as head_ps,
            tc.tile_pool(name="dram", bufs=1, space="DRAM") as dram,
        ):
            # ---- load persistent SBUF state
            esrc16 = res.tile([P, cfg.COLS * P // 16], dt.int16)
            edl = res.tile([P, cfg.COLS], dt.bfloat16)
            enrm = res.tile([P, cfg.COLS], dt.bfloat16)
            gloc = res.tile([P, NW], dt.bfloat16)
            dis2 = res.tile([P, NW], dt.bfloat16)
            w1a = res.tile([F0 + 1, F1], dt.bfloat16)
            w2a = res.tile([F1 + 1, F2], dt.bfloat16)
            w3a = res.tile([F2 + 1, F3], dt.bfloat16)
            fw1a = res.tile([F3 // 2, cfg.HID], dt.bfloat16)
            fw1b = res.tile([F3 // 2, cfg.HID], dt.bfloat16)
            fb1c = res.tile([cfg.HID, 1], dt.float32)
            fw2 = res.tile([cfg.HID, 1], dt.bfloat16)
            invc = res.tile([P, 1], dt.float32)
            b1r = res.tile([1, F1], dt.bfloat16)
            b2r = res.tile([1, F2], dt.bfloat16)
            b3r = res.tile([1, F3], dt.bfloat16)
            for sb, dr in ((esrc16, esrc_d), (edl, edl_d), (enrm, enrm_d),
                           (gloc, gloc_d), (dis2, dis2_d), (w1a, w1a_d), (w2a, w2a_d),
                           (w3a, w3a_d), (fb1c, fb1_d), (fw2, fw2_d),
                           (invc, invc_d)):
                nc.sync.dma_start(out=sb[:], in_=dr[:])
            nc.sync.dma_start(out=b1r[:], in_=w1a_d[F0:F0 + 1, :])
            nc.sync.dma_start(out=b2r[:], in_=w2a_d[F1:F1 + 1, :])
            nc.sync.dma_start(out=b3r[:], in_=w3a_d[F2:F2 + 1, :])
            nc.sync.dma_start(out=fw1a[:], in_=fw1_d[0:F3 // 2, :])
            nc.sync.dma_start(out=fw1b[:], in_=fw1_d[F3 // 2:, :])

            iota_i = res.tile([P, P], dt.int32)
            nc.gpsimd.iota(iota_i[:], pattern=[[1, P]], base=0, channel_multiplier=0)
            iota_b = res.tile([P, P], dt.bfloat16)
            nc.vector.tensor_copy(out=iota_b[:], in_=iota_i[:])
            ident = res.tile([P, P], dt.bfloat16)
            make_identity(nc, ident[:])
            ones1 = res.tile([1, P], dt.bfloat16)
            nc.vector.memset(ones1[:], 1.0)

            # ---- DRAM tables / bounce buffers (256B-row padded)
            h1s = dram.tile([cfg.SHARD, P], dt.bfloat16)
            h2s = dram.tile([cfg.SHARD, P], dt.bfloat16)
            h1t = dram.tile([cfg.NPAD, P], dt.bfloat16)
            h2t = dram.tile([cfg.NPAD, P], dt.bfloat16)
            pool_pt = dram.tile([P, F3], dt.float32)
            pool_rd = dram.tile([P, F3], dt.float32)

            pool_acc = pool_ps.tile([P, F3], dt.float32)
            callno = [0]

            def layer(tbl, F_in, F_out, waug, brow, self_src, shard_out,
                      ag=None, pre_ag=None):
                last = F_in == F2  # layer 3
                for chn in range(NCH):
                    pac = pa_ps.tile([FMAX, CW * P], dt.float32, tag="pa",
                                     name="pa")
                    pas = [pac[:F_in, wl * P:(wl + 1) * P] for wl in range(CW)]
                    for g in range(NG):
                        if pre_ag is not None and chn == 0 and g == NG // 2:
                            # groups 0/1 read table segment 0 only; emit the
                            # previous layer's seg-1 AllGather here so the Pool
                            # stream keeps issuing gathers while it lands
                            pre_ag()
                        call0 = (chn * NG + g) * CALL_COLS
                        fat = fatp.tile([P, CALL_COLS, P], dt.bfloat16,
                                        tag="fat", name="fat")
                        nc.gpsimd.dma_gather(
                            out_ap=fat[:],
                            in_ap=tbl[g * GRP:(g + 1) * GRP, :],
                            idxs_ap=esrc16[:, call0 * 8:(call0 + CALL_COLS) * 8],
                            num_idxs=CALL_IDX,
                            num_idxs_reg=CALL_IDX,
                            elem_size=P,
                            single_packet=False,
                            queue_num=callno[0] % NQ,
                        )
                        callno[0] += 1
                        cs = slice(call0, call0 + CALL_COLS)
                        mw = mwp.tile([P, CALL_COLS, FMAX], dt.bfloat16,
                                      tag="mw", name="mw")[:, :, :F_in]
                        nc.vector.tensor_tensor(
                            out=mw[:], in0=fat[:, :, :F_in],
                            in1=enrm[:, cs, None].broadcast_to([P, CALL_COLS, F_in]),
                            op=OP.mult)
                        S = sp.tile([P, CALL_COLS, P], dt.bfloat16, tag="S", name="S")
                        nc.vector.tensor_tensor(
                            out=S[:],
                            in0=edl[:, cs, None].broadcast_to([P, CALL_COLS, P]),
                            in1=iota_b[:, None, :].broadcast_to([P, CALL_COLS, P]),
                            op=OP.is_equal)
                        for wloc in range(CW):
                            for t in range(TG):
                                c = wloc * TG + t
                                nc.tensor.matmul(
                                    out=pas[wloc][:], lhsT=mw[:, c, :],
                                    rhs=S[:, c, :],
                                    start=(g == 0 and t == 0), stop=False)
                    for wloc in range(CW):
                        w = chn * CW + wloc
                        # self-loop term: pa[f,d] += h[d,f] * dis2[d] via a
                        # matmul with a diagonal rhs (local shard rows)
                        hw_t = work.tile([P, FMAX], dt.bfloat16, tag="hw",
                                         name="hw_t")[:, :F_in]
                        nc.sync.dma_start(out=hw_t[:],
                                          in_=self_src[w * P:(w + 1) * P, :F_in])
                        Dd = sp.tile([P, P], dt.bfloat16, tag="Dd", name="Dd")
                        nc.vector.tensor_tensor(
                            out=Dd[:], in0=ident[:],
                            in1=dis2[:, w:w + 1].broadcast_to([P, P]), op=OP.mult)
                        nc.tensor.matmul(out=pas[wloc][:], lhsT=hw_t[:], rhs=Dd[:],
                                         start=False, stop=True)
                        aggT = work.tile([FMAX, P], dt.bfloat16, tag="aggT",
                                         name="aggT")[:F_in]
                        pa = pas[wloc]
                        nc.scalar.copy(out=aggT[:], in_=pa[:])
                        p2 = p2_ps.tile([P, F3], dt.float32, tag="p2",
                                        name="p2")[:, :F_out]
                        nc.tensor.matmul(out=p2[:], lhsT=aggT[:], rhs=waug[:F_in, :],
                                         start=True, stop=False)
                        nc.tensor.matmul(out=p2[:], lhsT=ones1[:], rhs=brow[:],
                                         start=False, stop=True)
                        h = work.tile([P, F3], dt.bfloat16, tag="h",
                                      name="h")[:, :F_out]
                        nc.scalar.activation(h[:], p2[:], AF.Relu)
                        if not last:
                            nc.sync.dma_start(
                                out=shard_out[w * P:(w + 1) * P, :F_out], in_=h[:])
                        else:
                            Sg = sp.tile([P, P], dt.bfloat16, tag="Sg", name="Sg")
                            nc.vector.tensor_tensor(
                                out=Sg[:],
                                in0=gloc[:, w:w + 1].broadcast_to([P, P]),
                                in1=iota_b[:], op=OP.is_equal)
                            nc.tensor.matmul(out=pool_acc[:], lhsT=Sg[:], rhs=h[:],
                                             start=(w == 0), stop=(w == NW - 1))
                    if ag is not None and chn == 9:
                        # emitted well after the seg-0 windows (chunks 0-6) so
                        # the in-order Pool stream reaches it with the data
                        # dependency already satisfied (no Q7 stall)
                        ag(0)

            SEG = cfg.SHARD // 2
            HSEG = cfg.n_cores * SEG

            def seg_allgather(shard, table):
                def ag(sgi):
                    nc.gpsimd.collective_compute(
                        "AllGather", mybir.AluOpType.bypass, replica_groups=rg,
                        ins=[shard[sgi * SEG:(sgi + 1) * SEG, :].opt()],
                        outs=[table[sgi * HSEG:(sgi + 1) * HSEG, :].opt()])
                return ag

            ag1 = seg_allgather(h1s, h1t)
            ag2 = seg_allgather(h2s, h2t)
            layer(xt_d, F0, F1, w1a, b1r, xs_d, h1s, ag=ag1)
            layer(h1t, F1, F2, w2a, b2r, h1s, h2s, ag=ag2,
                  pre_ag=lambda: ag1(1))
            layer(h2t, F2, F3, w3a, b3r, h2s, None, pre_ag=lambda: ag2(1))

            # ---- pooling partial -> AllReduce -> mean
            psb = work.tile([P, F3], dt.float32, tag="psb")
            nc.scalar.copy(out=psb[:], in_=pool_acc[:])
            nc.sync.dma_start(out=pool_pt[:], in_=psb[:])
            nc.gpsimd.collective_compute(
                "AllReduce", mybir.AluOpType.add, replica_groups=rg,
                ins=[pool_pt.opt()], outs=[pool_rd.opt()])
            poolr = work.tile([P, F3], dt.float32, tag="poolr")
            nc.sync.dma_start(out=poolr[:], in_=pool_rd[:])
            pooled = work.tile([P, F3], dt.bfloat16, tag="pooled")
            nc.scalar.activation(pooled[:], poolr[:], AF.Copy, scale=invc[:])

            # ---- head: z1 = relu(pooled @ fW1 + fb1); z2 = z1 @ fW2 + fb2
            ptA_ps = head_ps.tile([F3 // 2, P], dt.bfloat16, tag="pt")
            nc.tensor.transpose(out=ptA_ps[:], in_=pooled[:, :F3 // 2], identity=ident[:])
            ptA = work.tile([F3 // 2, P], dt.bfloat16, tag="ptA")
            nc.scalar.copy(out=ptA[:], in_=ptA_ps[:])
            ptB_ps = head_ps.tile([F3 // 2, P], dt.bfloat16, tag="pt")
            nc.tensor.transpose(out=ptB_ps[:], in_=pooled[:, F3 // 2:], identity=ident[:])
            ptB = work.tile([F3 // 2, P], dt.bfloat16, tag="ptB")
            nc.scalar.copy(out=ptB[:], in_=ptB_ps[:])

            z1_ps = head_ps.tile([cfg.HID, P], dt.float32, tag="z1")
            nc.tensor.matmul(out=z1_ps[:], lhsT=fw1a[:], rhs=ptA[:], start=True, stop=False)
            nc.tensor.matmul(out=z1_ps[:], lhsT=fw1b[:], rhs=ptB[:], start=False, stop=True)
            z1 = work.tile([cfg.HID, P], dt.bfloat16, tag="z1s")
            nc.scalar.activation(z1[:], z1_ps[:], AF.Relu, bias=fb1c[:])

            z2_ps = head_ps.tile([1, P], dt.float32, tag="z2")
            nc.tensor.matmul(out=z2_ps[:], lhsT=fw2[:], rhs=z1[:], start=True, stop=True)
            z2 = work.tile([1, P], dt.float32, tag="z2s")
            nc.scalar.activation(z2[:], z2_ps[:], AF.Copy, bias=float(fb2))
            # softmax over a width-1 axis == 1.0 for finite logits
            outs = work.tile([1, P], dt.float32, tag="outs")
            nc.vector.tensor_tensor(out=outs[:], in0=z2[:], in1=z2[:], op=OP.is_equal)
            nc.sync.dma_start(out=out_d[:], in_=outs[:])

    nc.compile()
    return nc


# ---------------------------------------------------------------- run

_CACHE = {}


def _get_nc(cfg, fb2):
    key = (tuple(cfg.F), cfg.NW, cfg.TG, cfg.SHARD, fb2)
    if key not in _CACHE:
        _CACHE[key] = build_bass(cfg, fb2)
    return _CACHE[key]


def make_in_maps(cfg, inp):
    cores, xt, wts, fb2 = build_host_data(cfg, inp)
    in_maps = []
    for c in range(cfg.n_cores):
        m = dict(xt=xt, **cores[c], **wts)
        in_maps.append(m)
    return in_maps, fb2


def derive_cfg(inputs):
    """Auto-size TG to the densest (dst window, src group) cell."""
    cfg = CFG
    srcA = np.asarray(inputs["edge_index"][0]).astype(np.int64).ravel()
    dstA = np.asarray(inputs["edge_index"][1]).astype(np.int64).ravel()
    SEG = cfg.SHARD // 2
    vv = np.arange(cfg.NPAD, dtype=np.int64)
    vc, vr = vv // cfg.SHARD, vv % cfg.SHARD
    vs = vr // SEG
    remap = vs * (cfg.n_cores * SEG) + vc * SEG + (vr - vs * SEG)
    cell = (dstA // P) * cfg.NG + remap[srcA] // cfg.GRP
    ccnt = np.bincount(cell, minlength=(cfg.NPAD // P) * cfg.NG)
    need = max(1, int(-(-ccnt.max() // P)))
    if need != cfg.TG:
        cfg = GCNConfig(**{**cfg.__dict__, "TG": need})
    return cfg


def kernel(**inputs):
    cfg = derive_cfg(inputs)
    in_maps, fb2 = make_in_maps(cfg, inputs)
    nc = _get_nc(cfg, fb2)
    from concourse.bass_utils import run_bass_kernel_spmd
    res = run_bass_kernel_spmd(nc, in_maps, core_ids=list(range(cfg.n_cores)))
    out = np.asarray(res.results[0]["out"]).reshape(P)[:cfg.G]
    return out.reshape(cfg.G, 1).astype(np.float32)
